# revision 35
# baseline (speedup 1.0000x reference)
"""Trainium2 Bass kernel for the BDH block (LN + neuron gating + causal RoPE
attention + permuted-reshape z @ encoder + residual + LN).

Sharding (8 NeuronCores): 2-way data parallel on batch x 4-way tensor
parallel. Within each 4-core group, attention is sharded by head pairs
(2 heads/core, all-reduce on attn_out), and the neuron/encoder stages are
sharded on a 1024-wide slice of each head's 4096 neurons. The final
z @ encoder partial sums are combined with a ReduceScatter; each core does
the residual + final LN on its 256-row shard and the host concatenates.

Perf structure:
  - the two giant neuron GEMMs (x/y decoders) run in fp8-e4m3 with
    perf_mode=DoubleRow (2x PE throughput, K=256 per call); weights are
    host-scaled by 64 (power of two, folded back through the encoder) to
    stay in e4m3 range. z @ encoder stays bf16 for the error budget.
  - the attention pipeline (q/k/v/scores/softmax/Wo) is all-bf16.
  - all bulk weight streams use p-major host layouts so each DMA is one
    contiguous run per partition (descriptor-light); enc/xres stream on
    the gpsimd queue, dependency-gated so they never starve the
    latency-critical sync-queue loads.
  - AR1 is split per T-half and overlapped with the second half of
    attention plus K_STASH pre-computed x-neuron blocks; AR2 is chunked
    per 128-row head block with the residual+final-LN inlined two chunks
    behind, so only the last chunk's collective is exposed.
"""

import numpy as np

B, T, D, H = 2, 1024, 512, 8
N = 32768
HD = D // H          # 64
NH = N // H          # 4096
EPS = 1e-5

N_CORES = 8
TPG = 4              # cores per data-parallel group
HPC = 2              # heads per core (attention sharding)
NSL = NH // TPG      # 1024: per-core slice of each head's neurons
KT = D // 128        # 4 k-tiles over D
TT = T // 128        # 8 t-tiles
V = H                # 8: t-residue factor in the permuting reshape
UP = T // V          # 128: u per head-row-block
NLB = NSL // 128     # 8 sub-blocks per head slice
K_STASH = 24         # x-neuron blocks computed during the attention all-reduce
TSH = T // TPG       # 256: per-core rows of the final output shard

_RUNNER = None


FP8 = True           # fp8e4m3 DoubleRow for the x/y-neuron matmuls
WSCALE = 64.0        # host scale on decoder_x/y before fp8 cast (power of 2)


def _host_shards(inputs):
    """Build the 8 per-core input maps from the full inputs."""
    import ml_dtypes
    bf16 = ml_dtypes.bfloat16
    f8 = ml_dtypes.float8_e4m3

    x = np.asarray(inputs["x"], dtype=np.float32)
    Wq = np.asarray(inputs["Wq"], dtype=np.float32)
    Wk = np.asarray(inputs["Wk"], dtype=np.float32)
    Wv = np.asarray(inputs["Wv"], dtype=np.float32)
    Wo = np.asarray(inputs["Wo"], dtype=np.float32)
    dx = np.asarray(inputs["decoder_x"], dtype=np.float32)
    dy = np.asarray(inputs["decoder_y"], dtype=np.float32)
    enc = np.asarray(inputs["encoder"], dtype=np.float32)

    # rope tables in deinterleaved-row layout [128, T] (2 heads stacked; both
    # head slots share the same frequency table)
    inv_freq = 1.0 / (10000.0 ** (np.arange(0, HD, 2, dtype=np.float32) / HD))
    freqs = np.arange(T, dtype=np.float32)[:, None] * inv_freq[None, :]  # [T, 32]
    cos_t = np.cos(freqs).T  # [32, T]
    sin_t = np.sin(freqs).T
    c64 = np.concatenate([cos_t, cos_t], axis=0)            # [64, T]
    s64 = np.concatenate([-sin_t, sin_t], axis=0)           # [64, T]
    cos2 = np.concatenate([c64, c64], axis=0).astype(np.float32)  # [128, T]
    sin2 = np.concatenate([s64, s64], axis=0).astype(np.float32)

    # column permutations for q/k weight slices (deinterleave + swap)
    deint = np.concatenate([np.arange(0, HD, 2), np.arange(1, HD, 2)])  # [64]
    swap = np.concatenate([np.arange(32, 64), np.arange(0, 32)])        # [64]
    perm = np.concatenate([deint, HD + deint])                           # [128]
    perm_s = np.concatenate([deint[swap], HD + deint[swap]])

    # causal masks for the transposed-scores diagonal band, [4, 128, 512]
    masks = np.zeros((4, 128, 512), dtype=np.float32)
    ii = np.arange(128)[:, None]
    jj = np.arange(512)[None, :]
    for c in range(4):
        q = jj // 128
        jloc = jj % 128
        masks[c] = np.where(q < c, 0.0, np.where(q == c, (ii <= jloc).astype(np.float32), 1.0))

    ident = np.eye(128, dtype=np.float32)

    enc_r = enc.reshape(V, NH, D)

    in_maps = []
    for c in range(N_CORES):
        b = c // TPG
        r = c % TPG
        rows = slice(r * 128, (r + 1) * 128)   # q/k/v weight rows (2 heads)
        wqT = Wq[rows, :].T.copy()             # [512, 128]
        wkT = Wk[rows, :].T.copy()
        wvT = Wv[rows, :].T.copy()
        woT = Wo[:, rows].T.copy()             # [128, 512]
        dx_c = dx[:, :, r * NSL:(r + 1) * NSL].transpose(1, 0, 2).reshape(D, H * NSL)
        dy_c = dy[:, :, r * NSL:(r + 1) * NSL].transpose(1, 0, 2).reshape(D, H * NSL)
        # re-layout so each 128-column tile is one contiguous [128, 512] DMA:
        # dx2[nt, p, k*128+c] = dx_c[k*128+p, nt*128+c]
        dx_c = dx_c.reshape(KT, 128, H * NSL // 128, 128).transpose(2, 1, 0, 3).reshape(H * NSL // 128, 128, D)
        dy_c = dy_c.reshape(KT, 128, H * NSL // 128, 128).transpose(2, 1, 0, 3).reshape(H * NSL // 128, 128, D)
        enc_c = enc_r[:, r * NSL:(r + 1) * NSL, :].reshape(V * NSL, D)
        in_maps.append({
            "x_in": np.ascontiguousarray(x[b]),
            "x_res": np.ascontiguousarray(x[b, r * TSH:(r + 1) * TSH, :]),
            "wall": np.ascontiguousarray(np.concatenate(
                [wqT[:, perm], wqT[:, perm_s], wkT[:, perm], wkT[:, perm_s], wvT],
                axis=1)).astype(bf16),
            "woT": np.ascontiguousarray(woT).astype(bf16),
            "wvb": np.ascontiguousarray(
                wvT.reshape(KT, 128, 128).transpose(1, 0, 2)).astype(bf16),
            # p-major layouts: every weight fetch is one contiguous run per
            # partition (descriptor-light DMA)
            "dy_in": np.ascontiguousarray(
                dy_c.transpose(1, 0, 2) * (WSCALE if FP8 else 1.0)).astype(
                    f8 if FP8 else bf16),
            "dx_in": np.ascontiguousarray(
                dx_c.transpose(1, 0, 2) * (WSCALE if FP8 else 1.0)).astype(
                    f8 if FP8 else bf16),
            # with fp8, xr/yr each carry a WSCALE factor; fold the combined
            # 1/WSCALE^2 into encoder (exact: power of 2, bf16 is scale-free)
            "enc_in": np.ascontiguousarray(
                enc_c.reshape(V, NLB, 128, D).transpose(2, 1, 0, 3)
                / (WSCALE * WSCALE if FP8 else 1.0)).astype(bf16),
            "cossin": np.ascontiguousarray(np.concatenate([cos2, sin2], axis=1)),
            "masks_in": masks.astype(bf16),
            "ident": ident,
            "ones_in": np.ones((128, 64), dtype=np.float32),
        })
    return in_maps


def _build_program(collectives=True, n_devices=None, repeat_all=1,
                   use_rs=False, use_bc=True, use_pstr=False, n_stash=K_STASH,
                   cc1=True, cc2=True, bfcc=True, houter=True, chunk_ar2=True,
                   ar2_chunks=8, ar1_chunks=2, fp8=FP8):
    from contextlib import ExitStack

    import concourse.bacc as bacc
    import concourse.tile as tile
    from concourse import mybir

    f32 = mybir.dt.float32
    f32r = mybir.dt.float32r
    bf16 = mybir.dt.bfloat16
    f8 = mybir.dt.float8e4
    ndt = f8 if fp8 else bf16      # dtype of neuron-matmul operands
    PM = mybir.MatmulPerfMode
    AF = mybir.ActivationFunctionType

    def nmm(ps_ap, w3, x3, nk=KT):
        """ps += w3.T @ x3 accumulated over nk k-tiles (dim1 of both APs);
        fp8 uses DoubleRow pairs."""
        if fp8:
            for kp in range(nk // 2):
                nc.tensor.matmul(ps_ap, w3[:, 2 * kp:2 * kp + 2, :],
                                 x3[:, 2 * kp:2 * kp + 2, :],
                                 start=(kp == 0), stop=(kp == nk // 2 - 1),
                                 perf_mode=PM.DoubleRow)
        else:
            for k in range(nk):
                nc.tensor.matmul(ps_ap, w3[:, k, :], x3[:, k, :],
                                 start=(k == 0), stop=(k == nk - 1))

    if n_devices is None:
        n_devices = N_CORES if collectives else 1
    nc = bacc.Bacc("TRN2", target_bir_lowering=False, debug=False,
                   num_devices=n_devices)
    groups = [[0, 1, 2, 3], [4, 5, 6, 7]]

    def din(name, shape, dt=f32r):
        return nc.dram_tensor(name, shape, dt, kind="ExternalInput").ap()

    x_in = din("x_in", [T, D], f32)
    x_res = din("x_res", [TSH, D], f32)
    wall_in = din("wall", [D, 5 * 128], bf16)
    woT = din("woT", [128, D], bf16)
    wvb_in = din("wvb", [128, KT, 128], bf16)
    NB = H * NSL // 128
    dy_in = din("dy_in", [128, NB, D], ndt)
    dx_in = din("dx_in", [128, NB, D], ndt)
    enc_in = din("enc_in", [128, NLB, V, D], bf16)
    cossin = din("cossin", [128, 2 * T], f32)
    masks_in = din("masks_in", [4, 128, 512], bf16)
    ident = din("ident", [128, 128], f32)
    ones_in = din("ones_in", [128, 64])

    y_out = nc.dram_tensor("y_out", [TSH if use_rs else T, D], f32,
                           kind="ExternalOutput").ap()

    with tile.TileContext(nc) as tc:
        with tc.tile_pool(name="const", bufs=1) as const, \
             tc.tile_pool(name="persist", bufs=1) as persist, \
             tc.tile_pool(name="dram", bufs=1, space="DRAM") as dram, \
             tc.tile_pool(name="stats", bufs=8) as stats:

            eps_t = const.tile([128, 1], f32)
            nc.vector.memset(eps_t[:], EPS)
            ones_sb = const.tile([128, 64], bf16)
            nc.vector.memset(ones_sb[:], 1.0)
            ident_sb = const.tile([128, 128], f32)
            nc.sync.dma_start(out=ident_sb[:], in_=ident[:])
            ident_bf = const.tile([128, 128], bf16)
            nc.vector.tensor_copy(out=ident_bf[:], in_=ident_sb[:])
            cos_sb = const.tile([128, T], f32)
            sin_sb = const.tile([128, T], f32)

            def ln_tile(out_ap, in_ap):
                st = stats.tile([128, 6], f32, tag="ln_st")
                nc.vector.bn_stats(out=st[:], in_=in_ap)
                mv = stats.tile([128, 2], f32, tag="ln_mv")
                nc.vector.bn_aggr(out=mv[:], in_=st[:])
                sd = stats.tile([128, 1], f32, tag="ln_sd")
                nc.scalar.activation(out=sd[:], in_=mv[:, 1:2], func=AF.Sqrt,
                                     bias=eps_t[:])
                rs = stats.tile([128, 1], f32, tag="ln_rs")
                nc.vector.reciprocal(out=rs[:], in_=sd[:])
                nc.vector.tensor_scalar(out=out_ap, in0=in_ap,
                                        scalar1=mv[:, 0:1], scalar2=rs[:],
                                        op0=mybir.AluOpType.subtract,
                                        op1=mybir.AluOpType.mult)

            def ln_batch(pairs):
                n = len(pairs)
                sts, mvs, sds, rss = [], [], [], []
                for _, in_ap in pairs:
                    st = stats.tile([128, 6], f32, tag="ln_st")
                    nc.vector.bn_stats(out=st[:], in_=in_ap)
                    sts.append(st)
                for st in sts:
                    mv = stats.tile([128, 2], f32, tag="ln_mv")
                    nc.vector.bn_aggr(out=mv[:], in_=st[:])
                    mvs.append(mv)
                for mv in mvs:
                    sd = stats.tile([128, 1], f32, tag="ln_sd")
                    nc.scalar.activation(out=sd[:], in_=mv[:, 1:2], func=AF.Sqrt,
                                         bias=eps_t[:])
                    sds.append(sd)
                for sd in sds:
                    rs = stats.tile([128, 1], f32, tag="ln_rs")
                    nc.vector.reciprocal(out=rs[:], in_=sd[:])
                    rss.append(rs)
                for (out_ap, in_ap), mv, rs in zip(pairs, mvs, rss):
                    nc.vector.tensor_scalar(out=out_ap, in0=in_ap,
                                            scalar1=mv[:, 0:1], scalar2=rs[:],
                                            op0=mybir.AluOpType.subtract,
                                            op1=mybir.AluOpType.mult)

            # persistent SBUF tensors (shared across repeats)
            xnT_bf = persist.tile([128, KT, T], ndt)     # neuron-matmul copy of LN(x).T
            lnT = persist.tile([128, KT, T], ndt)        # LN(attn) transposed
            xr_stash = persist.tile([128, max(n_stash, 1), T], bf16)
            if houter:
                enc_sb = persist.tile([128, NLB, V, D], bf16)
            else:
                out_sb = persist.tile([128, H, D], f32)

            for _rep in range(repeat_all):
              rep = f"r{_rep}"
              # attention-local SBUF, freed before stage D needs the space
              persB_ctx = ExitStack()
              persB = persB_ctx.enter_context(
                  tc.tile_pool(name=f"persB{rep}", bufs=1))
              xnT = persB.tile([128, KT, T], bf16, name="xnT")
              qrot = persB.tile([128, T], bf16, name="qrot")
              krot = persB.tile([128, T], bf16, name="krot")
              v_sb = persB.tile([128, TT, 2, 65], bf16, name="v_sb")
              nc.vector.memset(v_sb[:, :, :, 64:65], 1.0)
              avn = [persB.tile([64, T], bf16, name=f"avn{h}") for h in range(2)]
              wB1_ctx = ExitStack()
              wB1 = wB1_ctx.enter_context(
                  tc.tile_pool(name=f"wB1{rep}", bufs=1))
              wall = wB1.tile([128, KT, 5 * 128], bf16, tag="wall")
              # ------------- Stage A+B1: x load, LN, transpose, q/k/v ---------
              # B1's projections for the first T-half interleave with the
              # LN+transpose of the second half so the PE never waits on DVE
              with tc.tile_pool(name=f"stA{rep}", bufs=1) as stA, \
                   tc.tile_pool(name=f"psA{rep}", bufs=2, space="PSUM") as psA, \
                   tc.tile_pool(name=f"stB1{rep}", bufs=2) as stB1, \
                   tc.tile_pool(name=f"psB1{rep}", bufs=2, space="PSUM") as psB1, \
                   tc.tile_pool(name=f"psV{rep}", bufs=2, space="PSUM") as psV:
                xn_sb = stA.tile([128, TT, D], f32)
                xn_bf = stA.tile([128, TT, D], bf16)
                for i0, i1 in ((0, 1), (1, 4), (4, 8)):
                    nc.sync.dma_start(
                        out=xn_sb[:, i0:i1, :],
                        in_=x_in[i0 * 128:i1 * 128, :].rearrange(
                            "(i p) d -> p i d", p=128))
                    if i0 == 0:
                        # attention weights + rope tables stream while LN runs
                        nc.sync.dma_start(
                            out=wall[:],
                            in_=wall_in.rearrange("(k p) j -> p k j", p=128))
                        if _rep == 0:
                            nc.sync.dma_start(out=cos_sb[:], in_=cossin[:, 0:T])
                            nc.sync.dma_start(out=sin_sb[:], in_=cossin[:, T:2 * T])
                w_q = wall[:, :, 0:128]
                w_qs = wall[:, :, 128:256]
                w_k = wall[:, :, 256:384]
                w_ks = wall[:, :, 384:512]
                w_v = wall[:, :, 512:640]

                def tr_tile(i):
                    for k in range(KT):
                        ps_tr = psA.tile([128, 128], bf16, tag="tr")
                        nc.tensor.transpose(ps_tr[:],
                                            xn_bf[:, i, k * 128:(k + 1) * 128],
                                            ident_bf[:])
                        nc.vector.tensor_copy(out=xnT[:, k, i * 128:(i + 1) * 128],
                                              in_=ps_tr[:])
                        nc.scalar.activation(
                            out=xnT_bf[:, k, i * 128:(i + 1) * 128],
                            in_=ps_tr[:], func=AF.Copy)

                def b1_f(f):
                    tsl = slice(f * 512, (f + 1) * 512)
                    for (wa, wb, rot) in ((w_q, w_qs, qrot), (w_k, w_ks, krot)):
                        ps_a = psB1.tile([128, 512], f32, tag="ps_a")
                        ps_b = psB1.tile([128, 512], f32, tag="ps_b")
                        for k in range(KT):
                            nc.tensor.matmul(ps_a[:], wa[:, k, :], xnT[:, k, tsl],
                                             start=(k == 0), stop=(k == KT - 1))
                        for k in range(KT):
                            nc.tensor.matmul(ps_b[:], wb[:, k, :], xnT[:, k, tsl],
                                             start=(k == 0), stop=(k == KT - 1))
                        t1 = stB1.tile([128, 512], f32, tag="ropetmp1")
                        t2 = stB1.tile([128, 512], f32, tag="ropetmp2")
                        nc.vector.tensor_mul(out=t1[:], in0=ps_a[:],
                                             in1=cos_sb[:, tsl])
                        nc.vector.tensor_mul(out=t2[:], in0=ps_b[:],
                                             in1=sin_sb[:, tsl])
                        nc.vector.tensor_add(out=rot[:, tsl], in0=t1[:], in1=t2[:])

                def b1_v(i):
                    ps_v = psV.tile([128, 128], f32, tag="ps_v")
                    for k in range(KT):
                        nc.tensor.matmul(ps_v[:], xnT[:, k, i * 128:(i + 1) * 128],
                                         w_v[:, k, :],
                                         start=(k == 0), stop=(k == KT - 1))
                    for h in range(2):
                        nc.vector.tensor_copy(out=v_sb[:, i, h, 0:64],
                                              in_=ps_v[:, h * 64:(h + 1) * 64])

                ln_batch([(xn_bf[:, i, :], xn_sb[:, i, :]) for i in range(4)])
                for i in range(4):
                    tr_tile(i)
                ln_batch([(xn_bf[:, i, :], xn_sb[:, i, :]) for i in range(4, TT)])
                b1_f(0)
                for i in range(4):
                    b1_v(i)
                for i in range(4, TT):
                    tr_tile(i)
                b1_f(1)
                for i in range(4, TT):
                    b1_v(i)
                # hold back the bulk enc load until B1 output exists so it
                # doesn't steal DMA bandwidth from the critical stage-A loads:
                # the corner write forces a write-after-write dep on enc_sb
                nc.gpsimd.tensor_copy(out=enc_sb[0:1, 0, 0, 0:1],
                                      in_=v_sb[0:1, 0, 0, 0:1])

              wB1_ctx.close()

              # B2: scores, exp, denominators, attn @ v, Wo projection
              ccdt = bf16 if bfcc else f32
              ap_dram = dram.tile([T, D], ccdt, tag="ar1_in")
              ar1_out = dram.tile([T, D], ccdt, tag="ar1_out")
              den_dram = dram.tile([2, T], f32, tag="den")
              assert houter and n_stash % 2 == 0
              GS = 2
              while n_stash % (2 * GS) == 0 and GS < 8:
                  GS *= 2
              xctx = ExitStack()
              wX = xctx.enter_context(tc.tile_pool(name=f"wX{rep}", bufs=3))

              def fetch_stash(s0):
                  dx_t = wX.tile([128, GS, KT, 128], ndt, tag="dxs", name="dx_t")
                  nc.sync.dma_start(
                      out=dx_t[:].rearrange("p j k c -> p j (k c)"),
                      in_=dx_in[:, s0:s0 + GS, :])
                  return dx_t

              with tc.tile_pool(name=f"stB2{rep}", bufs=6) as stB2, \
                   tc.tile_pool(name=f"stB2b{rep}", bufs=2) as stB2b, \
                   tc.tile_pool(name=f"wB2{rep}", bufs=1) as wB2, \
                   tc.tile_pool(name=f"psS{rep}", bufs=4, space="PSUM") as psS, \
                   tc.tile_pool(name=f"psAv{rep}", bufs=1, space="PSUM") as psAv:
                psAp = psS
                masks_sb = wB2.tile([128, 4, 512], bf16, tag="masks")
                nc.sync.dma_start(out=masks_sb[:], in_=masks_in.rearrange("c p n -> p c n"))
                wo_h = [wB2.tile([64, D], bf16, tag=f"wo{h}", name=f"wo_h{h}")
                        for h in range(2)]
                for h in range(2):
                    nc.sync.dma_start(out=wo_h[h][:], in_=woT[h * 64:(h + 1) * 64, :])
                stash_tiles = {s0: fetch_stash(s0)
                               for s0 in range(0, min(n_stash, 3 * GS), GS)}
                nc.gpsimd.dma_start(out=enc_sb[:], in_=enc_in[:])

                for f in range(2):
                    tsl = slice(f * 512, (f + 1) * 512)
                    av_ps = [psAv.tile([65, 512], f32, tag=f"av{h}", name=f"av_ps{h}")
                             for h in range(2)]
                    np_tiles = 4 * f + 4
                    def acc_pair(p, es):
                        for h in range(2):
                            nc.tensor.matmul(av_ps[h][:], v_sb[:, p, h, :], es[h][:],
                                             start=(p == 0), stop=(p == np_tiles - 1))
                    pipe = []
                    for p in range(np_tiles):
                        cur = []
                        for h in range(2):
                            hsl = slice(h * 64, (h + 1) * 64)
                            s_ps = psS.tile([128, 512], f32, tag="s", name="s_ps")
                            nc.tensor.matmul(s_ps[:], krot[hsl, p * 128:(p + 1) * 128],
                                             qrot[hsl, tsl], start=True, stop=True)
                            e_sb = stB2.tile([128, 512], bf16, tag="exp", name="e_sb")
                            nc.scalar.activation(out=e_sb[:], in_=s_ps[:], func=AF.Exp)
                            cstar = p - 4 * f
                            if cstar >= 0:
                                nc.vector.tensor_mul(out=e_sb[:], in0=e_sb[:],
                                                     in1=masks_sb[:, cstar, :])
                            cur.append(e_sb)
                        pipe.append((p, cur))
                        if len(pipe) > 3:
                            pp, es = pipe.pop(0)
                            acc_pair(pp, es)
                    for pp, es in pipe:
                        acc_pair(pp, es)
                    for h in range(2):
                        dr = stB2b.tile([1, 512], bf16, tag="denrow")
                        with nc.allow_low_precision(reason="f32r is bit-identical to f32"):
                            nc.vector.reciprocal(out=dr[:], in_=av_ps[h][64:65, :])
                        if use_bc:
                            bc_ps = psAp.tile([128, 512], f32, tag="s", name="bc_ps")
                            nc.tensor.matmul(bc_ps[0:64, :], ones_sb[0:1, :], dr[:],
                                             start=True, stop=True)
                            bc_sb = stB2b.tile([64, 512], bf16, tag="bcsb")
                            nc.vector.tensor_copy(out=bc_sb[:], in_=bc_ps[0:64, :])
                            nc.vector.tensor_mul(out=avn[h][:, tsl],
                                                 in0=av_ps[h][0:64, :],
                                                 in1=bc_sb[:])
                        else:
                            nc.sync.dma_start(out=den_dram[h:h + 1, tsl], in_=dr[:])
                            av_sb = stB2b.tile([64, 512], f32, tag="avsb", name="av_sb")
                            nc.vector.tensor_copy(out=av_sb[:], in_=av_ps[h][0:64, :])
                            den_bc = stB2b.tile([64, 512], f32, tag="denbc", name="den_bc")
                            nc.gpsimd.dma_start(
                                out=den_bc[:],
                                in_=den_dram[h:h + 1, tsl].partition_broadcast(64))
                            nc.vector.tensor_mul(out=avn[h][:, tsl], in0=av_sb[:],
                                                 in1=den_bc[:])
                    for i in range(4 * f, 4 * f + 4):
                        ap_ps = psAp.tile([128, 512], f32, tag="s", name="ap_ps")
                        for h in range(2):
                            nc.tensor.matmul(ap_ps[:],
                                             avn[h][:, i * 128:(i + 1) * 128],
                                             wo_h[h][:], start=(h == 0),
                                             stop=(h == 1))
                        o_sb = stB2b.tile([128, 512], ccdt, tag="apout")
                        nc.vector.tensor_copy(out=o_sb[:], in_=ap_ps[:])
                        nc.sync.dma_start(out=ap_dram[i * 128:(i + 1) * 128, :],
                                          in_=o_sb[:])
                    # attention all-reduce, chunked per T-half: the first half
                    # reduces while the second half's scores are still on PE
                    rsl = slice(f * 512, (f + 1) * 512)
                    if collectives and cc1:
                        nc.gpsimd.collective_compute(
                            "AllReduce", mybir.AluOpType.add,
                            replica_groups=groups,
                            ins=[ap_dram[rsl, :].opt()],
                            outs=[ar1_out[rsl, :].opt()])
                    else:
                        nc.sync.dma_start(out=ar1_out[rsl, :],
                                          in_=ap_dram[rsl, :])

              # ---------------- Stage X: x-neuron stash (overlaps AR1) --------
              with tc.tile_pool(name=f"psX{rep}", bufs=4, space="PSUM") as psX:
                for gi, s0 in enumerate(range(0, n_stash, GS)):
                    dx_t = stash_tiles[s0] if s0 in stash_tiles \
                        else fetch_stash(s0)
                    for s in range(s0, s0 + GS):
                        for tb in range(2):
                            tsl = slice(tb * 512, (tb + 1) * 512)
                            mm_ps = psX.tile([128, 512], f32, tag="xmm")
                            nmm(mm_ps[:], dx_t[:, s - s0, :, :], xnT_bf[:, :, tsl])
                            nc.scalar.activation(out=xr_stash[:, s, tsl], in_=mm_ps[:],
                                                 func=AF.Relu)
              xctx.close()
              persB_ctx.close()

              rs_in = dram.tile([T, D], ccdt, tag="rs_in")
              rs_out = dram.tile([TSH, D], ccdt, tag="rs_out") if use_rs \
                  else dram.tile([T, D], ccdt, tag="rs_out")
              GRP = 4
              assert n_stash % GRP == 0
              grp_tiles = {}
              dctx = ExitStack()
              wPre = dctx.enter_context(tc.tile_pool(name=f"wPre{rep}", bufs=2))

              def fetch_group(s0, wD):
                  # dy-only inside the stash region, dy+dx beyond it; both
                  # contiguous per partition in the p-major host layout
                  if s0 + GRP <= n_stash:
                      gt = wD.tile([128, GRP, KT, 128], ndt, tag="dyp", name="gt")
                      nc.sync.dma_start(
                          out=gt[:].rearrange("p j k c -> p j (k c)"),
                          in_=dy_in[:, s0:s0 + GRP, :])
                  else:
                      gt = wD.tile([128, GRP, 2 * KT, 128], ndt, tag="dxyp", name="gt")
                      nc.sync.dma_start(
                          out=gt[:, :, 0:KT, :].rearrange("p j k c -> p j (k c)"),
                          in_=dy_in[:, s0:s0 + GRP, :])
                      nc.sync.dma_start(
                          out=gt[:, :, KT:2 * KT, :].rearrange("p j k c -> p j (k c)"),
                          in_=dx_in[:, s0:s0 + GRP, :])
                  grp_tiles[s0] = gt
                  return gt

              # ---------------- Stage C: LN(attn_out), transpose ----------------
              with tc.tile_pool(name=f"stC{rep}", bufs=1) as stC, \
                   tc.tile_pool(name=f"psC{rep}", bufs=2, space="PSUM") as psC:
                af_sb = stC.tile([128, TT, D], ccdt, tag="af")
                af2_sb = stC.tile([128, TT, D], bf16, tag="af2")
                for i0 in (0, 4):
                    nc.sync.dma_start(
                        out=af_sb[:, i0:i0 + 4, :],
                        in_=ar1_out[i0 * 128:(i0 + 4) * 128, :].rearrange(
                            "(i p) d -> p i d", p=128))
                for s0 in (0, GRP):
                    if s0 < H * NLB:
                        fetch_group(s0, wPre)
                for g0 in range(0, TT, 4):
                    ln_batch([(af2_sb[:, i, :], af_sb[:, i, :])
                              for i in range(g0, g0 + 4)])
                for i in range(TT):
                    if use_pstr:
                        ps_tr = psC.tile([128, 512], f32, tag="trc")
                        for k in range(KT):
                            nc.tensor.transpose(ps_tr[:, k * 128:(k + 1) * 128],
                                                af_sb[:, i, k * 128:(k + 1) * 128], ident_sb[:])
                        nc.vector.tensor_copy(
                            out=lnT[:, :, i * 128:(i + 1) * 128],
                            in_=ps_tr[:].rearrange("p (k c) -> p k c", k=KT))
                    else:
                        for k in range(KT):
                            ps_tr = psC.tile([128, 128], bf16, tag="trc")
                            nc.tensor.transpose(ps_tr[:], af2_sb[:, i, k * 128:(k + 1) * 128], ident_bf[:])
                            nc.vector.tensor_copy(out=lnT[:, k, i * 128:(i + 1) * 128], in_=ps_tr[:])

              # ---------------- Stage D: neurons, gate, z @ enc ----------------
              def neuron_block(sidx, j, nlb, z_ps, wD, actD, psMM, flush):
                  s0 = sidx - sidx % GRP
                  gt = grp_tiles[s0] if s0 in grp_tiles else fetch_group(s0, wD)
                  if sidx == s0:
                      nxt = s0 + 2 * GRP
                      if nxt < H * NLB and nxt not in grp_tiles:
                          fetch_group(nxt, wD)
                  if sidx == s0 + GRP - 1:
                      grp_tiles.pop(s0, None)
                  dy_t = gt[:, sidx - s0, 0:KT, :]
                  dx_t = None if sidx < n_stash else gt[:, sidx - s0, KT:2 * KT, :]
                  yr = actD.tile([128, T], bf16, tag="yr", name="yr")
                  for tb in range(2):
                      tsl = slice(tb * 512, (tb + 1) * 512)
                      mm_ps = psMM.tile([128, 512], f32, tag="mm", name="mm_ps")
                      nmm(mm_ps[:], dy_t, lnT[:, :, tsl])
                      nc.scalar.activation(out=yr[:, tsl], in_=mm_ps[:], func=AF.Relu)
                  z_sb = actD.tile([128, T], bf16, tag="z", name="z_sb")
                  if sidx < n_stash:
                      flush()
                      nc.vector.tensor_mul(out=z_sb[:], in0=xr_stash[:, sidx, :],
                                           in1=yr[:])
                  else:
                      xr = actD.tile([128, T], bf16, tag="xr", name="xr")
                      for tb in range(2):
                          tsl = slice(tb * 512, (tb + 1) * 512)
                          mm_ps = psMM.tile([128, 512], f32, tag="mm", name="mm_ps")
                          nmm(mm_ps[:], dx_t, xnT_bf[:, :, tsl])
                          if tb == 0:
                              flush()
                          nc.scalar.activation(out=xr[:, tsl], in_=mm_ps[:], func=AF.Relu)
                      nc.vector.tensor_mul(out=z_sb[:], in0=xr[:], in1=yr[:])
                  return z_sb

              if houter:
                with tc.tile_pool(name=f"wD{rep}", bufs=3) as wD, \
                     tc.tile_pool(name=f"actD{rep}", bufs=3) as actD, \
                     tc.tile_pool(name=f"psMM{rep}", bufs=6, space="PSUM") as psMM, \
                     tc.tile_pool(name=f"psZ{rep}", bufs=2, space="PSUM") as psZ:
                  pending = []
                  fin_inline = chunk_ar2 and not use_rs and ar2_chunks == H
                  if fin_inline:
                      xres_b = wD.tile([128, TT, D], f32, tag="xres", bufs=1)
                      nc.gpsimd.tensor_copy(out=xres_b[0:1, 0, 0:1],
                                            in_=xr_stash[0:1, 0, 0:1])
                      for i0 in range(0, TT, 4):
                          nc.gpsimd.dma_start(
                              out=xres_b[:, i0:i0 + 4, :],
                              in_=x_in[i0 * 128:(i0 + 4) * 128, :].rearrange(
                                  "(i p) d -> p i d", p=128))

                  def fin_chunk(i):
                      # residual + final LN for output rows [i*128, (i+1)*128),
                      # scheduled two AR2 chunks behind the producer so the
                      # collective never stalls the DVE/ACT queues
                      fo = actD.tile([128, D], ccdt, tag="fo", name="fo")
                      nc.sync.dma_start(out=fo[:],
                                        in_=rs_out[i * 128:(i + 1) * 128, :])
                      fo2 = actD.tile([128, D], f32, tag="fo2", name="fo2")
                      nc.vector.tensor_add(out=fo2[:], in0=xres_b[:, i, :],
                                           in1=fo[:])
                      ln_tile(fo2[:], fo2[:])
                      nc.sync.dma_start(out=y_out[i * 128:(i + 1) * 128, :],
                                        in_=fo2[:])

                  def flush():
                      while pending:
                          z_sb, nlb_p, z_ps_p = pending.pop()
                          zv = z_sb[:].rearrange("p (u v) -> p v u", v=V)
                          for v in range(V):
                              nc.tensor.matmul(z_ps_p[:], zv[:, v, :],
                                               enc_sb[:, nlb_p, v, :],
                                               start=(nlb_p == 0 and v == 0),
                                               stop=(nlb_p == NLB - 1 and v == V - 1))

                  for h in range(H):
                      z_ps = psZ.tile([128, 512], f32, tag="zacc")
                      for nlb in range(NLB):
                          sidx = h * NLB + nlb
                          z_sb = neuron_block(sidx, sidx, nlb, z_ps, wD, actD, psMM,
                                              flush)
                          pending.append((z_sb, nlb, z_ps))
                      flush()
                      ob = wD.tile([128, D], ccdt, tag="ob")
                      nc.vector.tensor_copy(out=ob[:], in_=z_ps[:])
                      nc.sync.dma_start(out=rs_in[h * 128:(h + 1) * 128, :], in_=ob[:])
                      hpc2 = H // ar2_chunks
                      if chunk_ar2 and not use_rs and (h + 1) % hpc2 == 0:
                          h0 = (h + 1 - hpc2) * 128
                          h1 = (h + 1) * 128
                          if collectives and cc2:
                              nc.gpsimd.collective_compute(
                                  "AllReduce", mybir.AluOpType.add,
                                  replica_groups=groups,
                                  ins=[rs_in[h0:h1, :].opt()],
                                  outs=[rs_out[h0:h1, :].opt()])
                          else:
                              nc.sync.dma_start(out=rs_out[h0:h1, :],
                                                in_=rs_in[h0:h1, :])
                      if fin_inline and h >= 2:
                          fin_chunk(h - 2)
                  if fin_inline:
                      fin_chunk(H - 2)
                      fin_chunk(H - 1)
              else:
                with tc.tile_pool(name=f"encD{rep}", bufs=2) as encD, \
                     tc.tile_pool(name=f"wD{rep}", bufs=3) as wD, \
                     tc.tile_pool(name=f"actD{rep}", bufs=3) as actD, \
                     tc.tile_pool(name=f"psMM{rep}", bufs=6, space="PSUM") as psMM, \
                     tc.tile_pool(name=f"psZ{rep}", bufs=2, space="PSUM") as psZ:
                  for nlb in range(NLB):
                    enc_t = encD.tile([128, V, D], bf16, tag="enc")
                    for v in range(V):
                        nc.sync.dma_start(
                            out=enc_t[:, v, :],
                            in_=enc_in[v * NSL + nlb * 128: v * NSL + (nlb + 1) * 128, :])
                    for h in range(H):
                        sidx = nlb * H + h
                        j = h * NLB + nlb
                        z_ps = psZ.tile([128, 512], f32, tag="zacc")
                        z_sb = neuron_block(sidx, j, nlb, z_ps, wD, actD, psMM)
                        zv = z_sb[:].rearrange("p (u v) -> p v u", v=V)
                        for v in range(V):
                            nc.tensor.matmul(z_ps[:], zv[:, v, :], enc_t[:, v, :],
                                             start=(v == 0), stop=(v == V - 1))
                        if nlb == 0:
                            nc.vector.tensor_copy(out=out_sb[:, h, :], in_=z_ps[:])
                        else:
                            nc.vector.tensor_add(out=out_sb[:, h, :], in0=out_sb[:, h, :],
                                                 in1=z_ps[:])
                  for h in range(H):
                    if bfcc:
                        ob = wD.tile([128, D], bf16, tag="ob")
                        nc.vector.tensor_copy(out=ob[:], in_=out_sb[:, h, :])
                        nc.sync.dma_start(out=rs_in[h * 128:(h + 1) * 128, :], in_=ob[:])
                    else:
                        nc.sync.dma_start(out=rs_in[h * 128:(h + 1) * 128, :], in_=out_sb[:, h, :])

              dctx.close()
              chunked_done = houter and chunk_ar2 and not use_rs
              if (collectives and cc2) and not chunked_done:
                  kind = "ReduceScatter" if use_rs else "AllReduce"
                  nc.gpsimd.collective_compute(
                      kind, mybir.AluOpType.add, replica_groups=groups,
                      ins=[rs_in.opt()], outs=[rs_out.opt()])
              elif not (collectives and cc2) and not chunked_done:
                  nc.sync.dma_start(out=rs_out.opt(),
                                    in_=rs_in[0:TSH, :] if use_rs else rs_in.opt())

              # ---------------- Final: residual + LN ----------------
              fin_done = houter and chunk_ar2 and not use_rs and ar2_chunks == H
              n_fin = 0 if fin_done else (TSH if use_rs else T) // 128
              with tc.tile_pool(name=f"stF{rep}", bufs=3) as stF:
                if n_fin:
                    xsrc = x_res if use_rs else x_in
                    xres_b = stF.tile([128, n_fin, D], f32, tag="xres", bufs=1)
                    for i0 in range(0, n_fin, 4):
                        i1 = min(i0 + 4, n_fin)
                        nc.sync.dma_start(
                            out=xres_b[:, i0:i1, :],
                            in_=xsrc[i0 * 128:i1 * 128, :].rearrange(
                                "(i p) d -> p i d", p=128))
                for i in range(n_fin):
                    fo = stF.tile([128, D], ccdt, tag="fo")
                    nc.sync.dma_start(out=fo[:], in_=rs_out[i * 128:(i + 1) * 128, :])
                    fo2 = stF.tile([128, D], f32, tag="fo2")
                    nc.vector.tensor_add(out=fo2[:], in0=xres_b[:, i, :], in1=fo[:])
                    ln_tile(fo2[:], fo2[:])
                    nc.sync.dma_start(out=y_out[i * 128:(i + 1) * 128, :], in_=fo2[:])

    nc.compile()
    return nc


class _Runner:
    """Compile once, jit once, execute many times."""

    def __init__(self, **build_kwargs):
        import jax
        import numpy as _np
        from jax.sharding import Mesh, PartitionSpec
        from jax.experimental.shard_map import shard_map
        from concourse import bass2jax, mybir

        self.jax = jax
        nc = _build_program(**build_kwargs)
        self.nc = nc
        bass2jax.install_neuronx_cc_hook()

        in_names, out_names, out_avals, zero_outs = [], [], [], []
        pn = nc.partition_id_tensor.name if nc.partition_id_tensor else None
        for alloc in nc.m.functions[0].allocations:
            if not isinstance(alloc, mybir.MemoryLocationSet):
                continue
            name = alloc.memorylocations[0].name
            if alloc.kind == "ExternalInput":
                if name != pn:
                    in_names.append(name)
            elif alloc.kind == "ExternalOutput":
                out_names.append(name)
                shape = tuple(alloc.tensor_shape)
                dtype = mybir.dt.np(alloc.dtype)
                out_avals.append(jax.core.ShapedArray(shape, dtype))
                zero_outs.append(_np.zeros(shape, dtype))
        self.in_names, self.out_names = in_names, out_names
        self.zero_outs = zero_outs
        n_params = len(in_names)
        all_in = in_names + out_names + ([pn] if pn else [])

        def _body(*args):
            operands = list(args)
            if pn is not None:
                operands.append(bass2jax.partition_id_tensor())
            outs = bass2jax._bass_exec_p.bind(
                *operands, out_avals=tuple(out_avals), in_names=tuple(all_in),
                out_names=tuple(out_names), lowering_input_output_aliases=(),
                sim_require_finite=True, sim_require_nnan=True, nc=nc)
            return tuple(outs)

        devices = jax.devices()[:N_CORES]
        mesh = Mesh(np.asarray(devices), ("core",))
        n_all = n_params + len(out_names)
        self.fn = jax.jit(
            shard_map(_body, mesh=mesh,
                      in_specs=(PartitionSpec("core"),) * n_all,
                      out_specs=(PartitionSpec("core"),) * len(out_names),
                      check_rep=False),
            keep_unused=True)
        self.sharding = jax.sharding.NamedSharding(mesh, PartitionSpec("core"))
        self.out_avals = out_avals

    def device_args(self, in_maps):
        concat_in = [np.concatenate([m[nm] for m in in_maps], axis=0)
                     for nm in self.in_names]
        concat_zero = [np.zeros((N_CORES * z.shape[0], *z.shape[1:]), z.dtype)
                       for z in self.zero_outs]
        return [self.jax.device_put(a, self.sharding)
                for a in concat_in + concat_zero]

    def run(self, dev_args):
        outs = self.fn(*dev_args)
        self.jax.block_until_ready(outs)
        return outs

    def results(self, outs):
        per_core = []
        for c in range(N_CORES):
            per_core.append({
                nm: np.asarray(outs[i]).reshape(N_CORES, *self.out_avals[i].shape)[c]
                for i, nm in enumerate(self.out_names)})
        return per_core


def _get_runner():
    global _RUNNER
    if _RUNNER is None:
        _RUNNER = _Runner()
    return _RUNNER


def kernel(**inputs):
    import time as _time

    in_maps = _host_shards(inputs)
    last_exc = None
    for attempt in range(3):
        try:
            runner = _get_runner()
            outs = runner.run(runner.device_args(in_maps))
            res = runner.results(outs)
            if res[0]["y_out"].shape[0] == T:
                out = np.stack([res[0]["y_out"], res[TPG]["y_out"]], axis=0)
            else:
                out = np.stack(
                    [np.concatenate([res[b * TPG + r]["y_out"] for r in range(TPG)],
                                    axis=0) for b in range(B)], axis=0)
            return out.astype(np.float32)
        except Exception as exc:  # transient device/tunnel hiccups: retry once or twice
            last_exc = exc
            global _RUNNER
            _RUNNER = None
            _time.sleep(3.0)
    raise last_exc

